# revision 1
# baseline (speedup 1.0000x reference)
"""GRU-D style GRUI encoder kernel for Trainium2 (Bass/Tile), 8 NeuronCores.

Strategy: data-parallel over batch B=256 across 8 cores (32 sequences/core).
Per core everything is kept in a transposed layout [hidden-on-partitions,
batch-on-free] so the recurrence's matmuls use the (stationary) weights as
lhsT and the state as the streaming rhs, with no per-step transposes.

  fused tile F[p, k*32 + b] = v[k*128 + p, b]   (H=256 -> 2 partition tiles)

Precompute (x-part GEMMs + temporal decay beta) is done per 64-step chunk
into SBUF and overlaps with the recurrence of the previous chunk.
"""

import numpy as np
import ml_dtypes
from contextlib import ExitStack

import concourse.bass as bass
import concourse.bacc as bacc
import concourse.tile as tile
from concourse import mybir
from concourse.bass_utils import run_bass_kernel_spmd
from concourse.masks import make_identity

B, T, D, H = 256, 512, 128, 256
NCORES = 8
BL = B // NCORES          # 32 sequences per core
C = 64                    # recurrence chunk (steps)
NCHUNK = T // C
GSTEPS = 16               # steps per precompute GEMM group (N = 16*32 = 512)

FP32 = mybir.dt.float32
BF16 = mybir.dt.bfloat16
AF = mybir.ActivationFunctionType

_cache = {}


def _build():
    nc = bacc.Bacc("TRN2", target_bir_lowering=False, debug=False,
                   num_devices=NCORES)

    xT = nc.dram_tensor("xT", [D, T * BL], BF16, kind="ExternalInput")
    dTs = nc.dram_tensor("dTs", [D, T * BL], BF16, kind="ExternalInput")
    wx_rmu_d = nc.dram_tensor("wx_rmu", [D, 2 * H], BF16, kind="ExternalInput")
    wx_h_d = nc.dram_tensor("wx_h", [D, H], BF16, kind="ExternalInput")
    wtd_d = nc.dram_tensor("wtd", [D, H], BF16, kind="ExternalInput")
    whr0_d = nc.dram_tensor("wh_rmu0", [128, 2 * H], BF16, kind="ExternalInput")
    whr1_d = nc.dram_tensor("wh_rmu1", [128, 2 * H], BF16, kind="ExternalInput")
    whh0_d = nc.dram_tensor("wh_h0", [128, H], BF16, kind="ExternalInput")
    whh1_d = nc.dram_tensor("wh_h1", [128, H], BF16, kind="ExternalInput")
    b_rmu_d = nc.dram_tensor("b_rmu", [128, 4], FP32, kind="ExternalInput")
    b_h_d = nc.dram_tensor("b_h", [128, 2], FP32, kind="ExternalInput")
    nb_td_d = nc.dram_tensor("nb_td", [128, 2], FP32, kind="ExternalInput")
    out_d = nc.dram_tensor("hT_out", [128, 2 * BL], FP32, kind="ExternalOutput")

    with ExitStack() as ctx:
        tc = ctx.enter_context(tile.TileContext(nc))
        wpool = ctx.enter_context(tc.tile_pool(name="weights", bufs=1))
        xpool = ctx.enter_context(tc.tile_pool(name="xin", bufs=2))
        gxpool = ctx.enter_context(tc.tile_pool(name="gx", bufs=2))
        pre_ps = ctx.enter_context(tc.tile_pool(name="pre_ps", bufs=2, space="PSUM"))
        r_ps = ctx.enter_context(tc.tile_pool(name="r_ps", bufs=2, space="PSUM"))
        mu_ps = ctx.enter_context(tc.tile_pool(name="mu_ps", bufs=2, space="PSUM"))
        h_ps = ctx.enter_context(tc.tile_pool(name="h_ps", bufs=2, space="PSUM"))
        spool = ctx.enter_context(tc.tile_pool(name="state", bufs=3))

        # --- weights / constants into SBUF ---
        wx_rmu = wpool.tile([128, 2 * H], BF16)
        nc.sync.dma_start(wx_rmu, wx_rmu_d[:, :])
        wx_h = wpool.tile([128, H], BF16)
        nc.sync.dma_start(wx_h, wx_h_d[:, :])
        wtd = wpool.tile([128, H], BF16)
        nc.sync.dma_start(wtd, wtd_d[:, :])
        whr = []
        for k, dtens in enumerate((whr0_d, whr1_d)):
            t_ = wpool.tile([128, 2 * H], BF16, tag=f"whr{k}")
            nc.sync.dma_start(t_, dtens[:, :])
            whr.append(t_)
        whh = []
        for k, dtens in enumerate((whh0_d, whh1_d)):
            t_ = wpool.tile([128, H], BF16, tag=f"whh{k}")
            nc.sync.dma_start(t_, dtens[:, :])
            whh.append(t_)
        b_rmu = wpool.tile([128, 4], FP32)
        nc.sync.dma_start(b_rmu, b_rmu_d[:, :])
        b_h = wpool.tile([128, 2], FP32)
        nc.sync.dma_start(b_h, b_h_d[:, :])
        nb_td = wpool.tile([128, 2], FP32)
        nc.sync.dma_start(nb_td, nb_td_d[:, :])
        ident = wpool.tile([128, 128], BF16)
        make_identity(nc, ident)

        # Touch the bias tiles from DVE once so later TensorScalarPtr copies
        # don't carry a DMA wait (walrus rejects TSP with 2 sync waits).
        scratch = wpool.tile([128, 4], FP32, tag="scratch")
        nc.vector.tensor_copy(scratch, b_rmu)
        scratch2 = wpool.tile([128, 2], FP32, tag="scratch2")
        nc.vector.tensor_copy(scratch2, b_h)

        # initial state bh(0) = beta(0) * h0 = 0
        hb = spool.tile([128, 2 * BL], BF16, tag="hb")
        nc.vector.memset(hb, 0.0)

        for c in range(NCHUNK):
            xch = xpool.tile([128, C * BL], BF16, tag="xch")
            nc.sync.dma_start(xch, xT[:, c * C * BL:(c + 1) * C * BL])
            dch = xpool.tile([128, C * BL], BF16, tag="dch")
            nc.sync.dma_start(dch, dTs[:, c * C * BL:(c + 1) * C * BL])

            gxr = gxpool.tile([128, C, 4 * BL], BF16, tag="gxr")
            gxh = gxpool.tile([128, C, 2 * BL], BF16, tag="gxh")
            bet = gxpool.tile([128, C, 2 * BL], BF16, tag="bet")

            for g in range(C // GSTEPS):
                nsl = slice(g * GSTEPS * BL, (g + 1) * GSTEPS * BL)
                tsl = slice(g * GSTEPS, (g + 1) * GSTEPS)
                for m in range(4):
                    ps = pre_ps.tile([128, GSTEPS * BL], FP32, tag="ps")
                    nc.tensor.matmul(ps, wx_rmu[:, m * 128:(m + 1) * 128],
                                     xch[:, nsl], start=True, stop=True)
                    nc.vector.tensor_scalar_add(
                        gxr[:, tsl, m * BL:(m + 1) * BL],
                        ps.rearrange("p (t b) -> p t b", b=BL),
                        b_rmu[:, m:m + 1])
                for m in range(2):
                    ps = pre_ps.tile([128, GSTEPS * BL], FP32, tag="ps")
                    nc.tensor.matmul(ps, wx_h[:, m * 128:(m + 1) * 128],
                                     xch[:, nsl], start=True, stop=True)
                    nc.vector.tensor_scalar_add(
                        gxh[:, tsl, m * BL:(m + 1) * BL],
                        ps.rearrange("p (t b) -> p t b", b=BL),
                        b_h[:, m:m + 1])
                for m in range(2):
                    ps = pre_ps.tile([128, GSTEPS * BL], FP32, tag="ps")
                    nc.tensor.matmul(ps, wtd[:, m * 128:(m + 1) * 128],
                                     dch[:, nsl], start=True, stop=True)
                    # exp(-(z + b)) = exp(-z + (-b));  beta = min(result, 1)
                    nc.scalar.activation(
                        bet[:, tsl, m * BL:(m + 1) * BL],
                        ps.rearrange("p (t b) -> p t b", b=BL),
                        AF.Exp, bias=nb_td[:, m:m + 1], scale=-1.0)
            nc.vector.tensor_scalar_min(
                bet.rearrange("p t b -> p (t b)"),
                bet.rearrange("p t b -> p (t b)"), 1.0)

            # ---- recurrence over this chunk ----
            for i in range(C):
                t = c * C + i
                last = (t == T - 1)

                psr = r_ps.tile([128, 2 * BL], FP32, tag="psr")
                psm = mu_ps.tile([128, 2 * BL], FP32, tag="psm")
                psh = h_ps.tile([128, 2 * BL], FP32, tag="psh")

                # inject precomputed x-parts (+bias) into PSUM
                nc.tensor.matmul(psr, ident, gxr[:, i, 0:2 * BL],
                                 start=True, stop=False)
                nc.tensor.matmul(psm, ident, gxr[:, i, 2 * BL:4 * BL],
                                 start=True, stop=False)
                nc.tensor.matmul(psh, ident, gxh[:, i, :],
                                 start=True, stop=False)

                if not last:
                    # p = beta(t+1) * bh   (off critical path)
                    p_t = spool.tile([128, 2 * BL], BF16, tag="p")
                    nc.gpsimd.tensor_mul(p_t, bet[:, i, :], hb)

                # r gates first (they gate the critical path)
                for m in range(2):
                    for k in range(2):
                        nc.tensor.matmul(
                            psr[:, m * BL:(m + 1) * BL],
                            whr[k][:, m * 128:(m + 1) * 128],
                            hb[:, k * BL:(k + 1) * BL],
                            start=False, stop=(m == 1 and k == 1))
                r_t = spool.tile([128, 2 * BL], BF16, tag="r")
                nc.scalar.activation(r_t, psr, AF.Sigmoid)

                for m in range(2):
                    for k in range(2):
                        nc.tensor.matmul(
                            psm[:, m * BL:(m + 1) * BL],
                            whr[k][:, (m + 2) * 128:(m + 3) * 128],
                            hb[:, k * BL:(k + 1) * BL],
                            start=False, stop=(m == 1 and k == 1))
                mu_t = spool.tile([128, 2 * BL], BF16, tag="mu")
                nc.scalar.activation(mu_t, psm, AF.Sigmoid)

                rh_t = spool.tile([128, 2 * BL], BF16, tag="rh")
                nc.vector.tensor_mul(rh_t, r_t, hb)

                for m in range(2):
                    for k in range(2):
                        nc.tensor.matmul(
                            psh[:, m * BL:(m + 1) * BL],
                            whh[k][:, m * 128:(m + 1) * 128],
                            rh_t[:, k * BL:(k + 1) * BL],
                            start=False, stop=(m == 1 and k == 1))
                hhat_t = spool.tile([128, 2 * BL], BF16, tag="hh")
                nc.scalar.activation(hhat_t, psh, AF.Tanh)

                d_t = spool.tile([128, 2 * BL], BF16, tag="d")
                nc.vector.tensor_tensor(d_t, hhat_t, hb,
                                        op=mybir.AluOpType.subtract)

                if not last:
                    # w = beta(t+1) * mu  (off critical path)
                    w_t = spool.tile([128, 2 * BL], BF16, tag="w")
                    nc.gpsimd.tensor_mul(w_t, bet[:, i, :], mu_t)
                    e_t = spool.tile([128, 2 * BL], BF16, tag="e")
                    nc.vector.tensor_mul(e_t, w_t, d_t)
                    hb_new = spool.tile([128, 2 * BL], BF16, tag="hb")
                    nc.vector.tensor_add(hb_new, p_t, e_t)
                    hb = hb_new
                else:
                    e_t = spool.tile([128, 2 * BL], BF16, tag="e")
                    nc.vector.tensor_mul(e_t, mu_t, d_t)
                    hout = spool.tile([128, 2 * BL], FP32, tag="ho")
                    nc.vector.tensor_add(hout, hb, e_t)
                    nc.sync.dma_start(out_d[:, :], hout)

    nc.compile()
    return nc


def _prep_inputs(x, delta, W_mu, b_mu, W_r, b_r, W_h, b_h, W_td, b_td):
    bf = ml_dtypes.bfloat16
    # weights: first H rows act on h, last D rows act on x
    wh_rmu = np.concatenate([W_r[:H], W_mu[:H]], axis=1)      # [256, 512]
    wx_rmu = np.concatenate([W_r[H:], W_mu[H:]], axis=1)      # [128, 512]
    wh_h, wx_h = W_h[:H], W_h[H:]

    def pcol(v):  # [2*128] -> [128, 2] column-per-tile
        return np.ascontiguousarray(np.stack([v[:128], v[128:]], axis=1),
                                    dtype=np.float32)

    b_rmu_col = np.concatenate([b_r, b_mu])                    # [512]
    b_rmu_t = np.ascontiguousarray(
        np.stack([b_rmu_col[i * 128:(i + 1) * 128] for i in range(4)], axis=1),
        dtype=np.float32)                                      # [128, 4]

    shared = {
        "wx_rmu": np.ascontiguousarray(wx_rmu, dtype=bf),
        "wx_h": np.ascontiguousarray(wx_h, dtype=bf),
        "wtd": np.ascontiguousarray(W_td, dtype=bf),
        "wh_rmu0": np.ascontiguousarray(wh_rmu[:128], dtype=bf),
        "wh_rmu1": np.ascontiguousarray(wh_rmu[128:], dtype=bf),
        "wh_h0": np.ascontiguousarray(wh_h[:128], dtype=bf),
        "wh_h1": np.ascontiguousarray(wh_h[128:], dtype=bf),
        "b_rmu": b_rmu_t,
        "b_h": pcol(b_h),
        "nb_td": pcol(-b_td),
    }

    # delta shifted by one step: beta used at step t is beta(t+1)
    dshift = np.concatenate(
        [delta[:, 1:, :], np.zeros((B, 1, D), np.float32)], axis=1)

    in_maps = []
    for ci in range(NCORES):
        xs = x[ci * BL:(ci + 1) * BL]          # [32, 512, 128]
        ds = dshift[ci * BL:(ci + 1) * BL]
        # [BL, T, D] -> [D, T, BL] -> [D, T*BL]  (column t*BL + b)
        xt = np.ascontiguousarray(
            xs.transpose(2, 1, 0).reshape(D, T * BL), dtype=bf)
        dt_ = np.ascontiguousarray(
            ds.transpose(2, 1, 0).reshape(D, T * BL), dtype=bf)
        in_maps.append({"xT": xt, "dTs": dt_, **shared})
    return in_maps


def kernel(x, delta, W_mu, b_mu, W_r, b_r, W_h, b_h, W_td, b_td):
    args = tuple(np.asarray(a, dtype=np.float32) for a in
                 (x, delta, W_mu, b_mu, W_r, b_r, W_h, b_h, W_td, b_td))
    in_maps = _prep_inputs(*args)
    if "nc" not in _cache:
        _cache["nc"] = _build()
    res = run_bass_kernel_spmd(_cache["nc"], in_maps,
                               core_ids=list(range(NCORES)))
    out = np.empty((B, H), np.float32)
    for ci in range(NCORES):
        o = res.results[ci]["hT_out"]          # [128, 2*BL]
        for k in range(2):
            # o[p, k*BL + b] = h[b, k*128 + p]
            out[ci * BL:(ci + 1) * BL, k * 128:(k + 1) * 128] = \
                o[:, k * BL:(k + 1) * BL].T
    return out



# revision 5
# speedup vs baseline: 1.0365x; 1.0365x over previous
"""GRU-D style GRUI encoder kernel for Trainium2 (Bass/Tile), 8 NeuronCores.

Strategy: data-parallel over batch B=256 across 8 cores (32 sequences/core).
Per core everything is kept in a transposed layout [hidden-on-partitions,
batch-on-free] so the recurrence's matmuls use the (stationary) weights as
lhsT and the state as the streaming rhs, with no per-step transposes.

Recurrence restructured to shorten the cross-engine critical path:
  state hb_t = beta_t * h_t  (pre-decayed).  Per step:
    r   = sigmoid(Whr@hb + gxr)          mu = sigmoid(Whmu@hb + gxm)
    hh  = tanh(Whh@(r*hb) + gxh)
    w   = beta' * mu        (GpSimd, off critical path)
    a   = beta' - w         (GpSimd, off path)
    q   = a * hb            (GpSimd, off path)
    u   = w * hh            (DVE, on path)
    hb' = q + u             (DVE)
  and the next step's gate PSUM is accumulated speculatively:
    psr_{t+1} = gxr_{t+1} (identity inject) + Whr@q_t  [during tanh_t]
                + Whr@u_t                              [only waits on u]
so the only on-path work after tanh is u and the 4 Whr@u matmuls.
"""

import numpy as np
import ml_dtypes
from contextlib import ExitStack

import concourse.bass as bass
import concourse.bacc as bacc
import concourse.tile as tile
from concourse import mybir
from concourse.bass_utils import run_bass_kernel_spmd
from concourse.masks import make_identity

B, T, D, H = 256, 512, 128, 256
NCORES = 8
BL = B // NCORES          # 32 sequences per core
C = 64                    # recurrence chunk (steps)
NCHUNK = T // C
GSTEPS = 16               # steps per precompute GEMM group (N = 16*32 = 512)

FP32 = mybir.dt.float32
BF16 = mybir.dt.bfloat16
AF = mybir.ActivationFunctionType
OP = mybir.AluOpType

_cache = {}


def _build():
    nc = bacc.Bacc("TRN2", target_bir_lowering=False, debug=False,
                   num_devices=NCORES)

    xT = nc.dram_tensor("xT", [D, T * BL], BF16, kind="ExternalInput")
    dTs = nc.dram_tensor("dTs", [D, T * BL], BF16, kind="ExternalInput")
    wx_rmu_d = nc.dram_tensor("wx_rmu", [D, 2 * H], BF16, kind="ExternalInput")
    wx_h_d = nc.dram_tensor("wx_h", [D, H], BF16, kind="ExternalInput")
    wtd_d = nc.dram_tensor("wtd", [D, H], BF16, kind="ExternalInput")
    whr0_d = nc.dram_tensor("wh_rmu0", [128, 2 * H], BF16, kind="ExternalInput")
    whr1_d = nc.dram_tensor("wh_rmu1", [128, 2 * H], BF16, kind="ExternalInput")
    whh0_d = nc.dram_tensor("wh_h0", [128, H], BF16, kind="ExternalInput")
    whh1_d = nc.dram_tensor("wh_h1", [128, H], BF16, kind="ExternalInput")
    b_rmu_d = nc.dram_tensor("b_rmu", [128, 4], FP32, kind="ExternalInput")
    b_h_d = nc.dram_tensor("b_h", [128, 2], FP32, kind="ExternalInput")
    nb_td_d = nc.dram_tensor("nb_td", [128, 2], FP32, kind="ExternalInput")
    out_d = nc.dram_tensor("hT_out", [128, 2 * BL], FP32, kind="ExternalOutput")

    with ExitStack() as ctx:
        tc = ctx.enter_context(tile.TileContext(nc))
        wpool = ctx.enter_context(tc.tile_pool(name="weights", bufs=1))
        xpool = ctx.enter_context(tc.tile_pool(name="xin", bufs=2))
        gxpool = ctx.enter_context(tc.tile_pool(name="gx", bufs=2))
        pre_ps = ctx.enter_context(tc.tile_pool(name="pre_ps", bufs=2, space="PSUM"))
        r_ps = ctx.enter_context(tc.tile_pool(name="r_ps", bufs=2, space="PSUM"))
        mu_ps = ctx.enter_context(tc.tile_pool(name="mu_ps", bufs=2, space="PSUM"))
        h_ps = ctx.enter_context(tc.tile_pool(name="h_ps", bufs=2, space="PSUM"))
        spool = ctx.enter_context(tc.tile_pool(name="state", bufs=3))

        # --- weights / constants into SBUF ---
        wx_rmu = wpool.tile([128, 2 * H], BF16)
        nc.sync.dma_start(wx_rmu, wx_rmu_d[:, :])
        wx_h = wpool.tile([128, H], BF16)
        nc.sync.dma_start(wx_h, wx_h_d[:, :])
        wtd = wpool.tile([128, H], BF16)
        nc.sync.dma_start(wtd, wtd_d[:, :])
        whr = []
        for k, dtens in enumerate((whr0_d, whr1_d)):
            t_ = wpool.tile([128, 2 * H], BF16, tag=f"whr{k}")
            nc.sync.dma_start(t_, dtens[:, :])
            whr.append(t_)
        whh = []
        for k, dtens in enumerate((whh0_d, whh1_d)):
            t_ = wpool.tile([128, H], BF16, tag=f"whh{k}")
            nc.sync.dma_start(t_, dtens[:, :])
            whh.append(t_)
        b_rmu = wpool.tile([128, 4], FP32)
        nc.sync.dma_start(b_rmu, b_rmu_d[:, :])
        b_h = wpool.tile([128, 2], FP32)
        nc.sync.dma_start(b_h, b_h_d[:, :])
        nb_td = wpool.tile([128, 2], FP32)
        nc.sync.dma_start(nb_td, nb_td_d[:, :])
        ident = wpool.tile([128, 128], BF16)
        make_identity(nc, ident)

        # Touch the bias tiles from DVE once so later TensorScalarPtr copies
        # don't carry a DMA wait (walrus rejects TSP with 2 sync waits).
        scratch = wpool.tile([128, 4], FP32, tag="scratch")
        nc.vector.tensor_copy(scratch, b_rmu)
        scratch2 = wpool.tile([128, 2], FP32, tag="scratch2")
        nc.vector.tensor_copy(scratch2, b_h)

        # initial state hb(0) = beta(0) * h0 = 0
        hb = spool.tile([128, 2 * BL], BF16, tag="hb")
        nc.vector.memset(hb, 0.0)
        q_prev = None
        u_prev = None

        for c in range(NCHUNK):
            xch = xpool.tile([128, C * BL], BF16, tag="xch")
            nc.sync.dma_start(xch, xT[:, c * C * BL:(c + 1) * C * BL])
            dch = xpool.tile([128, C * BL], BF16, tag="dch")
            nc.sync.dma_start(dch, dTs[:, c * C * BL:(c + 1) * C * BL])

            gxr = gxpool.tile([128, C, 2 * BL], BF16, tag="gxr")
            gxm = gxpool.tile([128, C, 2 * BL], BF16, tag="gxm")
            gxh = gxpool.tile([128, C, 2 * BL], BF16, tag="gxh")
            bet = gxpool.tile([128, C, 2 * BL], BF16, tag="bet")

            for g in range(C // GSTEPS):
                nsl = slice(g * GSTEPS * BL, (g + 1) * GSTEPS * BL)
                tsl = slice(g * GSTEPS, (g + 1) * GSTEPS)
                for m in range(4):
                    ps = pre_ps.tile([128, GSTEPS * BL], FP32, tag="ps")
                    nc.tensor.matmul(ps, wx_rmu[:, m * 128:(m + 1) * 128],
                                     xch[:, nsl], start=True, stop=True)
                    dst = gxr if m < 2 else gxm
                    mm = m % 2
                    nc.vector.tensor_scalar_add(
                        dst[:, tsl, mm * BL:(mm + 1) * BL],
                        ps.rearrange("p (t b) -> p t b", b=BL),
                        b_rmu[:, m:m + 1])
                for m in range(2):
                    ps = pre_ps.tile([128, GSTEPS * BL], FP32, tag="ps")
                    nc.tensor.matmul(ps, wx_h[:, m * 128:(m + 1) * 128],
                                     xch[:, nsl], start=True, stop=True)
                    nc.vector.tensor_scalar_add(
                        gxh[:, tsl, m * BL:(m + 1) * BL],
                        ps.rearrange("p (t b) -> p t b", b=BL),
                        b_h[:, m:m + 1])
                for m in range(2):
                    ps = pre_ps.tile([128, GSTEPS * BL], FP32, tag="ps")
                    nc.tensor.matmul(ps, wtd[:, m * 128:(m + 1) * 128],
                                     dch[:, nsl], start=True, stop=True)
                    # exp(-(z + b)) = exp(-z + (-b)); beta = min(result, 1)
                    nc.scalar.activation(
                        bet[:, tsl, m * BL:(m + 1) * BL],
                        ps.rearrange("p (t b) -> p t b", b=BL),
                        AF.Exp, bias=nb_td[:, m:m + 1], scale=-1.0)
            nc.vector.tensor_scalar_min(
                bet.rearrange("p t b -> p (t b)"),
                bet.rearrange("p t b -> p (t b)"), 1.0)

            # ---- recurrence over this chunk ----
            for i in range(C):
                t = c * C + i
                last = (t == T - 1)

                psr = r_ps.tile([128, 2 * BL], FP32, tag="psr")
                psm = mu_ps.tile([128, 2 * BL], FP32, tag="psm")
                psh = h_ps.tile([128, 2 * BL], FP32, tag="psh")

                first = q_prev is None
                # -- PE prep block: runs during tanh_{t-1} --
                # psr_t = gxr_t + Whr@q_{t-1}; psm_t likewise; psh_t inject.
                nc.tensor.matmul(psr, ident, gxr[:, i, :],
                                 start=True, stop=first)
                if q_prev is not None:
                    for m in range(2):
                        for k in range(2):
                            nc.tensor.matmul(
                                psr[:, m * BL:(m + 1) * BL],
                                whr[k][:, m * 128:(m + 1) * 128],
                                q_prev[:, k * BL:(k + 1) * BL],
                                start=False, stop=False)
                nc.tensor.matmul(psm, ident, gxm[:, i, :],
                                 start=True, stop=first)
                if q_prev is not None:
                    for m in range(2):
                        for k in range(2):
                            nc.tensor.matmul(
                                psm[:, m * BL:(m + 1) * BL],
                                whr[k][:, (m + 2) * 128:(m + 3) * 128],
                                q_prev[:, k * BL:(k + 1) * BL],
                                start=False, stop=False)
                nc.tensor.matmul(psh, ident, gxh[:, i, :],
                                 start=True, stop=False)

                # -- complete psr with Whr@u_{t-1}: only waits on u --
                if u_prev is not None:
                    for m in range(2):
                        for k in range(2):
                            nc.tensor.matmul(
                                psr[:, m * BL:(m + 1) * BL],
                                whr[k][:, m * 128:(m + 1) * 128],
                                u_prev[:, k * BL:(k + 1) * BL],
                                start=False, stop=(m == 1 and k == 1))
                r_t = spool.tile([128, 2 * BL], BF16, tag="r")
                nc.scalar.activation(r_t, psr, AF.Sigmoid)

                if u_prev is not None:
                    for m in range(2):
                        for k in range(2):
                            nc.tensor.matmul(
                                psm[:, m * BL:(m + 1) * BL],
                                whr[k][:, (m + 2) * 128:(m + 3) * 128],
                                u_prev[:, k * BL:(k + 1) * BL],
                                start=False, stop=(m == 1 and k == 1))
                mu_t = spool.tile([128, 2 * BL], BF16, tag="mu")
                nc.scalar.activation(mu_t, psm, AF.Sigmoid)

                rh_t = spool.tile([128, 2 * BL], BF16, tag="rh")
                nc.vector.tensor_mul(rh_t, r_t, hb)

                for m in range(2):
                    for k in range(2):
                        nc.tensor.matmul(
                            psh[:, m * BL:(m + 1) * BL],
                            whh[k][:, m * 128:(m + 1) * 128],
                            rh_t[:, k * BL:(k + 1) * BL],
                            start=False, stop=(m == 1 and k == 1))

                if not last:
                    # off-path: w = beta' * mu ; a = beta' - w
                    w_t = spool.tile([128, 2 * BL], BF16, tag="w")
                    nc.gpsimd.tensor_mul(w_t, bet[:, i, :], mu_t)
                    a_t = spool.tile([128, 2 * BL], BF16, tag="a")
                    nc.gpsimd.tensor_tensor(a_t, bet[:, i, :], w_t,
                                            op=OP.subtract)

                hhat_t = spool.tile([128, 2 * BL], BF16, tag="hh")
                nc.scalar.activation(hhat_t, psh, AF.Tanh)

                if not last:
                    q_t = spool.tile([128, 2 * BL], BF16, tag="q")
                    nc.gpsimd.tensor_mul(q_t, a_t, hb)

                    u_t = spool.tile([128, 2 * BL], BF16, tag="u")
                    nc.vector.tensor_mul(u_t, w_t, hhat_t)
                    hb_new = spool.tile([128, 2 * BL], BF16, tag="hb")
                    nc.vector.tensor_add(hb_new, q_t, u_t)
                    hb = hb_new
                    q_prev = q_t
                    u_prev = u_t
                else:
                    d_t = spool.tile([128, 2 * BL], BF16, tag="d")
                    nc.vector.tensor_tensor(d_t, hhat_t, hb,
                                            op=OP.subtract)
                    e_t = spool.tile([128, 2 * BL], BF16, tag="e")
                    nc.vector.tensor_mul(e_t, mu_t, d_t)
                    hout = spool.tile([128, 2 * BL], FP32, tag="ho")
                    nc.vector.tensor_add(hout, hb, e_t)
                    nc.sync.dma_start(out_d[:, :], hout)

    nc.compile()
    return nc


def _prep_inputs(x, delta, W_mu, b_mu, W_r, b_r, W_h, b_h, W_td, b_td):
    bf = ml_dtypes.bfloat16
    # weights: first H rows act on h, last D rows act on x
    wh_rmu = np.concatenate([W_r[:H], W_mu[:H]], axis=1)      # [256, 512]
    wx_rmu = np.concatenate([W_r[H:], W_mu[H:]], axis=1)      # [128, 512]
    wh_h, wx_h = W_h[:H], W_h[H:]

    def pcol(v):  # [2*128] -> [128, 2] column-per-tile
        return np.ascontiguousarray(np.stack([v[:128], v[128:]], axis=1),
                                    dtype=np.float32)

    b_rmu_col = np.concatenate([b_r, b_mu])                    # [512]
    b_rmu_t = np.ascontiguousarray(
        np.stack([b_rmu_col[i * 128:(i + 1) * 128] for i in range(4)], axis=1),
        dtype=np.float32)                                      # [128, 4]

    shared = {
        "wx_rmu": np.ascontiguousarray(wx_rmu, dtype=bf),
        "wx_h": np.ascontiguousarray(wx_h, dtype=bf),
        "wtd": np.ascontiguousarray(W_td, dtype=bf),
        "wh_rmu0": np.ascontiguousarray(wh_rmu[:128], dtype=bf),
        "wh_rmu1": np.ascontiguousarray(wh_rmu[128:], dtype=bf),
        "wh_h0": np.ascontiguousarray(wh_h[:128], dtype=bf),
        "wh_h1": np.ascontiguousarray(wh_h[128:], dtype=bf),
        "b_rmu": b_rmu_t,
        "b_h": pcol(b_h),
        "nb_td": pcol(-b_td),
    }

    # delta shifted by one step: beta used at step t is beta(t+1)
    dshift = np.concatenate(
        [delta[:, 1:, :], np.zeros((B, 1, D), np.float32)], axis=1)

    in_maps = []
    for ci in range(NCORES):
        xs = x[ci * BL:(ci + 1) * BL]          # [32, 512, 128]
        ds = dshift[ci * BL:(ci + 1) * BL]
        # [BL, T, D] -> [D, T, BL] -> [D, T*BL]  (column t*BL + b)
        xt = np.ascontiguousarray(
            xs.transpose(2, 1, 0).reshape(D, T * BL), dtype=bf)
        dt_ = np.ascontiguousarray(
            ds.transpose(2, 1, 0).reshape(D, T * BL), dtype=bf)
        in_maps.append({"xT": xt, "dTs": dt_, **shared})
    return in_maps


def kernel(x, delta, W_mu, b_mu, W_r, b_r, W_h, b_h, W_td, b_td):
    args = tuple(np.asarray(a, dtype=np.float32) for a in
                 (x, delta, W_mu, b_mu, W_r, b_r, W_h, b_h, W_td, b_td))
    in_maps = _prep_inputs(*args)
    if "nc" not in _cache:
        _cache["nc"] = _build()
    res = run_bass_kernel_spmd(_cache["nc"], in_maps,
                               core_ids=list(range(NCORES)))
    out = np.empty((B, H), np.float32)
    for ci in range(NCORES):
        o = res.results[ci]["hT_out"]          # [128, 2*BL]
        for k in range(2):
            # o[p, k*BL + b] = h[b, k*128 + p]
            out[ci * BL:(ci + 1) * BL, k * 128:(k + 1) * 128] = \
                o[:, k * BL:(k + 1) * BL].T
    return out
